# revision 54
# baseline (speedup 1.0000x reference)
"""Bahdanau-attention kernel for Trainium2, 8 NeuronCores, data-parallel over batch.

Reference computation (per batch b):
    q = query @ Wq.T                      # (1, H)
    k = keys @ Wk.T                       # (S, H)
    energy = tanh(q + k) @ v              # (S,)
    weights = softmax(energy)             # (S,)
    context = weights @ keys              # (H,)
Returns (context (B,1,H), weights (B,1,S)).

Device strategy (per core, BL=8 batches):
  - Host pre-transposes keys -> keysT [BL, H, S] so DMA loads land as
    [h, s] tiles (h on partitions).  SWDGE DMA casts f32->bf16 in flight;
    the whole keysT shard (16 MB bf16) is cached in SBUF and HBM is read
    exactly once (memory roofline ~94 us/core).  DMA emission is interleaved
    with the compute blocks so the gpsimd queue alternates descriptor
    generation with broadcasts.
  - kT = Wk @ keysT tiles on PE (bf16, full rate), 1024-wide 2-bank PSUM
    tiles so each tanh is one wide ScalarE op (per-partition bias = qT col).
  - energy via PE with per-batch column-masked v ("vmask") accumulated into
    per-half-batch-group PSUM tiles [4, 512] -> each group's exp fires as
    soon as its half of the batch is done (halves the critical-path latency
    from "tile block" to "context unblocked", and shortens the final tail).
  - exp WITHOUT max subtraction (|energy| <= ||v||_1 ~ 13, safe in f32),
    ScalarE Exp with accum_out producing the softmax denominator partials.
  - context = sum_s p[s]*keysT[h,s]: p rows are staged to partition 0 with
    tiny DMAs, gpsimd partition_broadcast replicates them to 128 partitions,
    and DVE scalar_tensor_tensor does the fused mul+reduce over s into
    per-quarter partial columns; one strided reduce + 1/l multiply + PE
    transpose finish the epilogue.
"""

import sys

sys.path.insert(0, "/opt/trn_rl_repo")

import numpy as np
import ml_dtypes

B, S, H = 64, 4096, 256
NCORES = 8
BL = B // NCORES  # 8 batches per core
HC = 2  # h chunks of 128
NSQ = 4  # s quarters of 1024
SQ = S // NSQ  # 1024
NST = 8  # s tiles of 512
STILE = S // NST  # 512

_cache = {}

# test.py can flip this to get a profiled run
TRACE = False
LAST_EXEC_NS = None

# debug bisect flags
USE_PBCAST = True  # partition_broadcast + rl_t dma for ctx normalize
USE_TTR = True  # tensor_tensor_reduce context reduction
USE_TRANSPOSE = True  # PE-transpose epilogue


def _build():
    import concourse.bass as bass
    import concourse.mybir as mybir
    import concourse.tile as tile
    from concourse import bacc
    from contextlib import ExitStack

    f32 = mybir.dt.float32
    f32r = mybir.dt.float32r
    bf16 = mybir.dt.bfloat16
    AF = mybir.ActivationFunctionType
    ALU = mybir.AluOpType

    nc = bacc.Bacc()

    keysT = nc.declare_dram_parameter("keysT", [BL, H, S], f32, isOutput=False)
    wkT = nc.declare_dram_parameter("wkT", [H, H], bf16, isOutput=False)
    wqT = nc.declare_dram_parameter("wqT", [H, H], bf16, isOutput=False)
    queryT = nc.declare_dram_parameter("queryT", [H, BL], bf16, isOutput=False)
    vmask = nc.declare_dram_parameter("vmask", [HC, 128, BL * BL], bf16, isOutput=False)
    sel = nc.declare_dram_parameter("sel", [BL, BL * 128], bf16, isOutput=False)
    ident = nc.declare_dram_parameter("ident", [128, 128], bf16, isOutput=False)
    out_ctx = nc.declare_dram_parameter("out_ctx", [BL, H], f32, isOutput=True)
    out_w = nc.declare_dram_parameter("out_w", [BL, S], f32, isOutput=True)

    with tile.TileContext(nc) as tc, ExitStack() as ctx:
        const = ctx.enter_context(tc.tile_pool(name="const", bufs=1))
        kc_pool = ctx.enter_context(tc.tile_pool(name="kc", bufs=1))
        t_pool = ctx.enter_context(tc.tile_pool(name="t", bufs=6))
        scr_pool = ctx.enter_context(tc.tile_pool(name="scr", bufs=4))
        small = ctx.enter_context(tc.tile_pool(name="small", bufs=1))
        psum_mm = ctx.enter_context(tc.tile_pool(name="psum_mm", bufs=2, space="PSUM"))
        psum_e = ctx.enter_context(tc.tile_pool(name="psum_e", bufs=4, space="PSUM"))

        # ---- constant loads (HWDGE) ----
        wk_sb = const.tile([128, 2 * H], bf16, tag="wk")
        for ci in range(HC):
            nc.sync.dma_start(wk_sb[:, ci * H : (ci + 1) * H], wkT[ci * 128 : (ci + 1) * 128, :])
        wq_sb = const.tile([128, 2 * H], bf16, tag="wq")
        for ci in range(HC):
            nc.sync.dma_start(wq_sb[:, ci * H : (ci + 1) * H], wqT[ci * 128 : (ci + 1) * 128, :])
        qry_sb = const.tile([128, HC * BL], bf16, tag="qry")
        for ci in range(HC):
            nc.sync.dma_start(qry_sb[:, ci * BL : (ci + 1) * BL], queryT[ci * 128 : (ci + 1) * 128, :])
        vm_sb = const.tile([128, HC * BL * BL], bf16, tag="vm")
        for hc in range(HC):
            nc.sync.dma_start(vm_sb[:, hc * BL * BL : (hc + 1) * BL * BL], vmask[hc])
        sel_sb = const.tile([BL, BL * 128], bf16, tag="sel")
        nc.sync.dma_start(sel_sb[:], sel[:])
        id_sb = const.tile([128, 128], bf16, tag="ident")
        nc.sync.dma_start(id_sb[:], ident[:])

        # ---- qT[co] = Wq @ query^T : [128, BL] per output-h chunk ----
        qt_sb = const.tile([128, HC * BL], f32, tag="qt")
        for co in range(HC):
            qt_ps = psum_e.tile([128, BL], f32, tag="e")
            for ci in range(HC):
                nc.tensor.matmul(
                    qt_ps[:],
                    wq_sb[:, ci * H + co * 128 : ci * H + (co + 1) * 128],
                    qry_sb[:, ci * BL : (ci + 1) * BL],
                    start=(ci == 0),
                    stop=(ci == 1),
                )
            nc.vector.tensor_copy(qt_sb[:, co * BL : (co + 1) * BL], qt_ps[:])

        # ---- keysT cache, bf16 cast in flight (SWDGE, merged DMAs) ----
        # merged tiles per (hc, batch-half); one dma_start covers 4 batches.
        # DMA emission is interleaved with the sq blocks so the gpsimd queue
        # alternates descriptor-generation with p-row broadcasts.
        BH = BL // 2
        kc_m = [
            [
                kc_pool.tile([128, BH * S], bf16, tag=f"kcm_{hc}_{bh}", name=f"kcm_{hc}_{bh}")
                for bh in range(2)
            ]
            for hc in range(HC)
        ]

        def kcap(b, hc, sq, lo=0, width=SQ):
            bh, bi = divmod(b, BH)
            base = bi * S + sq * SQ + lo
            return kc_m[hc][bh][:, base : base + width]

        def emit_kc_dma(sq):
            for bh in range(2):
                for hc in range(HC):
                    dst = kc_m[hc][bh][:].rearrange("p (b s) -> p b s", b=BH)[
                        :, :, sq * SQ : (sq + 1) * SQ
                    ]
                    src = keysT[
                        bh * BH : (bh + 1) * BH,
                        hc * 128 : (hc + 1) * 128,
                        sq * SQ : (sq + 1) * SQ,
                    ].rearrange("b p s -> p b s")
                    nc.gpsimd.dma_start(dst, src)

        emit_kc_dma(0)
        emit_kc_dma(1)

        # softmax state, split into two half-batch groups (lo = b0-3, hi =
        # b4-7) so each group's exp fires as soon as ITS energy accumulation
        # stops -- halves the latency from "tile block starts" to "context
        # unblocked"
        G, GB = 2, BL // 2
        # joint p tiles (st 0-3, whole-batch exp), group p tiles (st 4-7)
        p_j = [
            small.tile([BL, STILE], bf16, tag=f"pj_{st}", name=f"pj_{st}")
            for st in range(4)
        ]
        p_g = [
            [
                small.tile([GB, STILE], bf16, tag=f"p_{st}_{g}", name=f"p_{st}_{g}")
                for g in range(G)
            ]
            for st in range(NST)
        ]
        l_j = small.tile([BL, 4], f32, tag="lj")
        # context partial slots: co-major columns i = co*BL + b, NSLOT slots
        # each: st0-3 (early, st-granular), sq2, st6, st7
        NSLOT = 7
        l_parts_g = [
            small.tile([GB, NST], f32, tag=f"lparts{g}", name=f"lparts{g}")
            for g in range(G)
        ]
        ctx_sb = small.tile([128, BL * HC * NSLOT], f32, tag="ctxchain")
        l_sum_g = [
            small.tile([GB, 1], f32, tag=f"lsum{g}", name=f"lsum{g}") for g in range(G)
        ]
        rl_g = [
            small.tile([GB, 1], f32, tag=f"rl{g}", name=f"rl{g}") for g in range(G)
        ]
        w_pool = ctx.enter_context(tc.tile_pool(name="w", bufs=2))
        # partition-0 staging of p rows for the gpsimd broadcast (st >= 4),
        # one tile per s-tile, 2 rotating slots
        p0_pool = ctx.enter_context(tc.tile_pool(name="p0", bufs=2))
        p0_st = {}

        def emit_ctx(b, co, slot, kc_ap, in1_ap, eng):
            scr = scr_pool.tile(
                [128, kc_ap.shape[1]], bf16, tag="scr", name=f"scr_{b}_{co}_{slot}"
            )
            cidx = (co * BL + b) * NSLOT + slot
            eng.scalar_tensor_tensor(
                out=scr[:],
                in0=kc_ap,
                scalar=1.0,
                in1=in1_ap,
                op0=ALU.mult,
                op1=ALU.mult,
                accum_out=ctx_sb[:, cidx : cidx + 1],
            )

        def stage_p0(st, g):
            # stage group g's p rows at partition 0 for the gpsimd broadcast
            for bi in range(GB):
                b = g * GB + bi
                nc.sync.dma_start(
                    p0_st[st][0:1, b * STILE : (b + 1) * STILE],
                    p_g[st][g][bi : bi + 1, :],
                )

        def emit_exp(sq, stl, g, e_ps):
            st = sq * 2 + stl
            # exp (no max subtraction; |energy| <= ||v||_1, safe in f32)
            nc.scalar.activation(
                p_g[st][g][:], e_ps[:], AF.Exp,
                accum_out=l_parts_g[g][:, st : st + 1],
            )
            if g == 0:
                p0_st[st] = p0_pool.tile(
                    [1, BL * STILE], bf16, tag="p0", name=f"p0_{st}"
                )
            stage_p0(st, g)

        nc.vector.memset(ctx_sb[:], 0.0)

        # ---- main pipeline ----
        for sq in range(NSQ):
            early = False  # PE-bcast hybrid regressed; pb everywhere
            if early:
                # joint accumulators [BL, STILE]: whole-batch exp; context via
                # PE selector-broadcast (gpsimd is busy generating the cast-DMA
                # descriptors at this point)
                e_ps = [
                    psum_e.tile([BL, STILE], f32, tag="e", name=f"eps_{sq}_{i}")
                    for i in range(2)
                ]
            else:
                # e_ps[stl][g]: per s-tile and half-batch group
                e_ps = [
                    [
                        psum_e.tile([GB, STILE], f32, tag="e", name=f"eps_{sq}_{i}_{g}")
                        for g in range(G)
                    ]
                    for i in range(2)
                ]
            for b in range(BL):
                g, bi = divmod(b, GB)
                for co in range(HC):
                    t = t_pool.tile([128, SQ], bf16, tag="t", name=f"t_{b}_{co}")
                    ktp = psum_mm.tile([128, SQ], f32, tag="mm", name=f"kt_{b}_{co}")
                    for stl in range(2):
                        for ci in range(HC):
                            nc.tensor.matmul(
                                ktp[:, stl * STILE : (stl + 1) * STILE],
                                wk_sb[:, ci * H + co * 128 : ci * H + (co + 1) * 128],
                                kcap(b, ci, sq, stl * STILE, STILE),
                                start=(ci == 0),
                                stop=(ci == 1),
                            )
                    # one wide tanh over both psum banks
                    nc.scalar.activation(
                        t[:], ktp[:], AF.Tanh,
                        bias=qt_sb[:, co * BL + b : co * BL + b + 1],
                    )
                    for stl in range(2):
                        if early:
                            nc.tensor.matmul(
                                e_ps[stl][:],
                                vm_sb[:, co * BL * BL + b * BL :
                                      co * BL * BL + (b + 1) * BL],
                                t[:, stl * STILE : (stl + 1) * STILE],
                                start=(b == 0 and co == 0),
                                stop=(b == BL - 1 and co == HC - 1),
                            )
                        else:
                            nc.tensor.matmul(
                                e_ps[stl][g][:],
                                vm_sb[:, co * BL * BL + b * BL + g * GB :
                                      co * BL * BL + b * BL + (g + 1) * GB],
                                t[:, stl * STILE : (stl + 1) * STILE],
                                start=(bi == 0 and co == 0),
                                stop=(bi == GB - 1 and co == HC - 1),
                            )
                if early and b == BL - 1:
                    for stl in range(2):
                        st = sq * 2 + stl
                        nc.scalar.activation(
                            p_j[st][:], e_ps[stl][:], AF.Exp,
                            accum_out=l_j[:, st : st + 1],
                        )
                if (not early) and bi == GB - 1:
                    # this group's energy is complete for both s-tiles
                    for stl in range(2):
                        emit_exp(sq, stl, g, e_ps[stl][g])
                    if sq == NSQ - 1:
                        # last quarter: context for this group can go right now
                        for stl in range(2):
                            st = sq * 2 + stl
                            for bb in range(g * GB, (g + 1) * GB):
                                pb_h = scr_pool.tile(
                                    [128, STILE], bf16, tag="pbh",
                                    name=f"pbh_{stl}_{bb}",
                                )
                                nc.gpsimd.partition_broadcast(
                                    pb_h[:],
                                    p0_st[st][0:1, bb * STILE : (bb + 1) * STILE],
                                )
                                for co in range(HC):
                                    emit_ctx(
                                        bb, co, 5 + stl,
                                        kcap(bb, co, sq, stl * STILE, STILE),
                                        pb_h[:], nc.vector,
                                    )
                        # weights-output for this group: l = sum of the joint
                        # (st0-3) and group (st4-7) partials
                        red_a = small.tile(
                            [GB, 1], f32, tag=f"reda{g}", name=f"red_a{g}"
                        )
                        nc.vector.reduce_sum(
                            red_a[:], l_parts_g[g][:], axis=mybir.AxisListType.X
                        )
                        nc.vector.reciprocal(rl_g[g][:], red_a[:])
                        for st in range(NST):
                            w_t = w_pool.tile(
                                [GB, STILE], f32, tag="w", name=f"w_{g}_{st}"
                            )
                            nc.vector.tensor_scalar_mul(
                                w_t[:], p_g[st][g][:], rl_g[g][:]
                            )
                            nc.sync.dma_start(
                                out_w[g * GB : (g + 1) * GB,
                                      st * STILE : (st + 1) * STILE],
                                w_t[:],
                            )
            if sq == NSQ - 1:
                # rl pairs for the context normalize: co-major [rl | rl] row
                rl2 = small.tile([1, BL * HC], f32, tag="rl2")
                for co in range(HC):
                    for g in range(G):
                        nc.sync.dma_start(
                            rl2[0:1, co * BL + g * GB : co * BL + (g + 1) * GB],
                            rl_g[g][:],
                        )
                rl_bc = small.tile([128, BL * HC], f32, tag="rlbc")
                nc.gpsimd.partition_broadcast(rl_bc[:], rl2[:])
            elif sq == NSQ - 2:
                for b in range(BL):
                    pb_sb = scr_pool.tile([128, SQ], bf16, tag="pb", name=f"pb_{sq}_{b}")
                    for stl in range(2):
                        nc.gpsimd.partition_broadcast(
                            pb_sb[:, stl * STILE : (stl + 1) * STILE],
                            p0_st[sq * 2 + stl][0:1, b * STILE : (b + 1) * STILE],
                        )
                    for co in range(HC):
                        emit_ctx(b, co, 4, kcap(b, co, sq), pb_sb[:], nc.vector)
            else:
                for b in range(BL):
                    pb_sb = scr_pool.tile([128, SQ], bf16, tag="pb", name=f"pbe_{sq}_{b}")
                    for stl in range(2):
                        nc.gpsimd.partition_broadcast(
                            pb_sb[:, stl * STILE : (stl + 1) * STILE],
                            p0_st[sq * 2 + stl][0:1, b * STILE : (b + 1) * STILE],
                        )
                    for co in range(HC):
                        emit_ctx(b, co, sq, kcap(b, co, sq), pb_sb[:], nc.vector)
            if sq + 2 < NSQ:
                emit_kc_dma(sq + 2)

        # ---- epilogue: reduce context partials, normalize by 1/l ----
        ctx_red = small.tile([128, BL * HC], f32, tag="ctxred")
        ctx_fin = small.tile([128, BL * HC], bf16, tag="ctxfin")
        nc.vector.reduce_sum(
            ctx_red[:],
            ctx_sb[:].rearrange("p (i q) -> p i q", q=NSLOT),
            axis=mybir.AxisListType.X,
        )
        nc.vector.tensor_mul(ctx_fin[:], ctx_red[:], rl_bc[:])
        if USE_TRANSPOSE:
            ctxT_ps = psum_e.tile([BL * HC, 128], bf16, tag="e")
            nc.tensor.matmul(ctxT_ps[:], ctx_fin[:], id_sb[:], is_transpose=True)
            ctxT_sb = small.tile([BL * HC, 128], f32, tag="ctxT")
            nc.vector.tensor_copy(ctxT_sb[:], ctxT_ps[:])
            for co in range(HC):
                nc.sync.dma_start(
                    out_ctx[:, co * 128 : (co + 1) * 128],
                    ctxT_sb[co * BL : (co + 1) * BL, :],
                )
        else:
            ctxT_sb = small.tile([BL * HC, 128], f32, tag="ctxT")
            nc.gpsimd.memset(ctxT_sb[:], 0.0)
            nc.sync.dma_start(
                out_ctx.rearrange("b (c h) -> (b c) h", c=HC), ctxT_sb[:]
            )

    nc.finalize()
    return nc


def kernel(query, keys, Wq, Wk, v):
    global LAST_EXEC_NS
    from concourse.bass_utils import run_bass_kernel_spmd

    if "nc" not in _cache:
        _cache["nc"] = _build()
    nc = _cache["nc"]

    query = np.asarray(query, dtype=np.float32)
    keys = np.asarray(keys, dtype=np.float32)
    Wq = np.asarray(Wq, dtype=np.float32)
    Wk = np.asarray(Wk, dtype=np.float32)
    v = np.asarray(v, dtype=np.float32)

    bf16 = ml_dtypes.bfloat16
    wkT_np = np.ascontiguousarray(Wk.T).astype(bf16)
    wqT_np = np.ascontiguousarray(Wq.T).astype(bf16)
    vmask_np = np.zeros((HC, 128, BL * BL), dtype=np.float32)
    for hc in range(HC):
        for b in range(BL):
            vmask_np[hc, :, b * BL + b] = v[hc * 128 : (hc + 1) * 128]
    vmask_np = vmask_np.astype(bf16)
    sel_np = np.zeros((BL, BL * 128), dtype=np.float32)
    for b in range(BL):
        sel_np[b, b * 128 : (b + 1) * 128] = 1.0
    sel_np = sel_np.astype(bf16)
    ident_np = np.eye(128, dtype=np.float32).astype(bf16)

    in_maps = []
    for i in range(NCORES):
        sl = slice(i * BL, (i + 1) * BL)
        keysT_np = np.ascontiguousarray(keys[sl].transpose(0, 2, 1))
        queryT_np = np.ascontiguousarray(query[sl, 0, :].T).astype(bf16)
        in_maps.append(
            {
                "keysT": keysT_np,
                "wkT": wkT_np,
                "wqT": wqT_np,
                "queryT": queryT_np,
                "vmask": vmask_np,
                "sel": sel_np,
                "ident": ident_np,
            }
        )

    res = run_bass_kernel_spmd(nc, in_maps, core_ids=list(range(NCORES)), trace=TRACE)
    LAST_EXEC_NS = res.exec_time_ns

    context = np.zeros((B, 1, H), dtype=np.float32)
    weights = np.zeros((B, 1, S), dtype=np.float32)
    for i in range(NCORES):
        sl = slice(i * BL, (i + 1) * BL)
        context[sl, 0, :] = res.results[i]["out_ctx"]
        weights[sl, 0, :] = res.results[i]["out_w"]
    return context, weights
